# revision 16
# baseline (speedup 1.0000x reference)
"""Trainium2 Bass kernel for AffineModulatedLatentODEFunc.

Layout: features on SBUF partitions, batch on the free dimension.
Conv layers are block-sparse matmuls over (position*64+channel) feature
vectors of length 1024 (16 spatial positions x 64 channels). conv1/conv2
run in fp8-e4m3 with DoubleRow perf mode; the fp8 weight pre-scale folds
into the gating vectors. Sin features are precomputed on the host.

Engine budget per 512-sample tile (ns, from the TRN2 cost model):
  ACT : cin tanh 4x1038 + conv tanh 4x1892 + exp 2x570 + G2/BB2 2x612
  DVE : conv mults 8x1192 + a-branch 2x658 + G1/BB1 2x658 + osb 658
        + one add-half per conv layer (fp16 4x mode)
  Pool: ELU stt-merge 2x~700 + one add-half per conv layer
  PE  : ~50 matmul streams (cin row-tiled 4-way, cout fp8-DR pairs)
ELU is computed as h = max(pre+b,0) + min(exp(pre+b),1), which needs one
ACT op (Exp straight off PSUM with folded bias) and lets W2 contract a
single h tensor (one matmul instead of two).
"""
import sys

if "/opt/trn_rl_repo" not in sys.path:
    sys.path.insert(0, "/opt/trn_rl_repo")

import numpy as np

import concourse.bacc as bacc
import concourse.mybir as mybir
import concourse.tile as tile
from concourse.alu_op_type import AluOpType
from concourse.bass_utils import run_bass_kernel_spmd

import ml_dtypes

F32 = mybir.dt.float32
F16 = mybir.dt.float16
FP8 = mybir.dt.float8e4
AFT = mybir.ActivationFunctionType
PM = mybir.MatmulPerfMode

NP16 = np.float16
NP8 = ml_dtypes.float8_e4m3

B = 32768
NCORES = 8
BL = B // NCORES          # 4096 samples per core
NT = 512                  # batch tile
NIT = BL // NT            # 8 iterations per core
TMAX = 31.41592653589793
SC = 16.0                 # fp8 conv-weight pre-scale (folded out via G)

OPTS = {
    "front_lead": 2,      # software-pipeline stages the front runs ahead
    "hh_bufs": 5, "gate_bufs": 6, "t_bufs": 3, "mlp_bufs": 4,
    "cps_bufs": 3, "mg_bufs": 2,
}

# chunk m covers spatial positions {2m, 2m+1}; all position-pairs of chunk m
# sit in grid row m//2, so chunk adjacency blocks form consecutive runs with
# an even count (4 or 6) -> pair them up for DoubleRow matmuls.
def _chunk_rows(m):
    return {(2 * m) // 4, (2 * m + 1) // 4}

BLOCKS_OF = {}
for m in range(8):
    kbs = [kb for kb in range(8)
           if any(abs(ri - ro) <= 1 for ri in _chunk_rows(kb)
                  for ro in _chunk_rows(m))]
    BLOCKS_OF[m] = sorted(kbs)

PAIRS_OF = {}          # m -> list of kb1 (pair = kb1, kb1+1)
PAIRLIST = []          # (m, kb1) in emission order
for m in range(8):
    kbs = BLOCKS_OF[m]
    assert len(kbs) % 2 == 0 and kbs == list(range(kbs[0], kbs[-1] + 1))
    ps = [kbs[i] for i in range(0, len(kbs), 2)]
    PAIRS_OF[m] = ps
    for kb1 in ps:
        PAIRLIST.append((m, kb1))
NPAIR = len(PAIRLIST)  # 20
PIDX = {mk: i for i, mk in enumerate(PAIRLIST)}


def _conv_feature_matrix(w):
    """(cout, cin, 3, 3) conv weight -> (16*cin, 16*cout) feature matrix."""
    ncout, ncin = w.shape[0], w.shape[1]
    E = np.zeros((16 * ncin, 16 * ncout), dtype=np.float64)
    for p_in in range(16):
        ii, jj = divmod(p_in, 4)
        for p_out in range(16):
            io, jo = divmod(p_out, 4)
            di, dj = ii - io + 1, jj - jo + 1
            if 0 <= di <= 2 and 0 <= dj <= 2:
                E[p_in * ncin:(p_in + 1) * ncin,
                  p_out * ncout:(p_out + 1) * ncout] = w[:, :, di, dj].T
    return E


def _pack_pairs(E, scale):
    """(1024,1024) feature matrix -> (128, NPAIR*256) DoubleRow-paired blocks."""
    cols = []
    for (m, kb1) in PAIRLIST:
        b0 = E[kb1 * 128:(kb1 + 1) * 128, m * 128:(m + 1) * 128]
        b1 = E[(kb1 + 1) * 128:(kb1 + 2) * 128, m * 128:(m + 1) * 128]
        cols.append(np.concatenate([b0, b1], axis=1))   # (128, 2*128)
    return (np.concatenate(cols, axis=1) * scale).astype(np.float64)


def _prepare(inputs):
    """Host-side weight folding + input layout prep. Pure numpy."""
    t = np.asarray(inputs["t"], np.float32)
    z = np.asarray(inputs["z"], np.float32)
    mu = np.asarray(inputs["mu"], np.float32)
    W1 = np.asarray(inputs["W1"], np.float32)
    b1 = np.asarray(inputs["b1"], np.float32)
    W2 = np.asarray(inputs["W2"], np.float32)
    b2 = np.asarray(inputs["b2"], np.float32)
    Wg = np.asarray(inputs["Wg"], np.float32)
    bg = np.asarray(inputs["bg"], np.float32)
    cin_w = np.asarray(inputs["cin_w"], np.float32)
    cin_b = np.asarray(inputs["cin_b"], np.float32)
    c1_w = np.asarray(inputs["c1_w"], np.float32)
    c1_b = np.asarray(inputs["c1_b"], np.float32)
    c2_w = np.asarray(inputs["c2_w"], np.float32)
    c2_b = np.asarray(inputs["c2_b"], np.float32)
    cout_w = np.asarray(inputs["cout_w"], np.float32)
    cout_b = np.asarray(inputs["cout_b"], np.float32)

    freqs = (1.0 / (np.float32(TMAX) ** np.arange(4, dtype=np.float32)))

    # ---- sin features on host: 16 sin rows then 16 cos rows, v-major f-minor
    V = np.stack([t, mu[:, 0], mu[:, 1], mu[:, 2]], axis=0)      # (4, B)
    args = V[:, None, :] * freqs[None, :, None]                  # (4, 4, B)
    x16 = np.concatenate(
        [np.sin(args).reshape(16, B), np.cos(args).reshape(16, B)], axis=0
    ).astype(NP16)                                               # (32, B)

    # device row -> reference feature row permutation
    perm = np.empty(32, np.int64)
    for q in range(16):
        perm[q] = 8 * (q // 4) + (q % 4)
        perm[16 + q] = 8 * (q // 4) + 4 + (q % 4)
    W1p = np.ascontiguousarray(W1[perm, :])                      # (32, 128)

    # ELU shift folds: h' = elu(pre+b)+1, so b2' = b2 - colsum(W2)
    b2p = b2 - W2.sum(axis=0)
    bgp = bg - Wg.sum(axis=0)

    g1c, g2c = Wg[:, 0:64], Wg[:, 64:128]
    bb1c, bb2c = Wg[:, 128:192], Wg[:, 192:256]
    bb1p = bb1c + g1c * c1_b[None, :]
    bb2p = bb2c + g2c * c2_b[None, :]
    s = 1.0 / SC
    wg_cat = np.concatenate(
        [g1c * s, g1c * s, g2c * s, g2c * s, bb1p, bb1p, bb2p, bb2p], axis=1
    ).astype(np.float32)                                         # (128, 512)
    bgp1, bgp2 = bgp[0:64], bgp[64:128]
    bbb1 = bgp[128:192] + bgp1 * c1_b
    bbb2 = bgp[192:256] + bgp2 * c2_b
    gvec = np.stack(
        [np.concatenate([bgp1, bgp1]) * s, np.concatenate([bgp2, bgp2]) * s,
         np.concatenate([bbb1, bbb1]), np.concatenate([bbb2, bbb2])], axis=1
    ).astype(np.float32)                                         # (128, 4)

    # ---- z as raw 16 rows (padding rows are zero -> dropped from win)
    z16 = np.ascontiguousarray(z.reshape(B, 16).T)               # (16, B)

    # ---- cin effective weights, row-tiled layout: chunk m at partition
    # rows 32*(m%4)..+16, column block m//4.  (16, per-chunk 128) each.
    win = np.zeros((16, 8 * 128), np.float64)
    for mchunk in range(8):
        for pp in range(2):
            p = 2 * mchunk + pp
            io, jo = divmod(p, 4)
            for p_in in range(16):
                ii, jj = divmod(p_in, 4)
                di, dj = ii - io + 1, jj - jo + 1
                if 0 <= di <= 2 and 0 <= dj <= 2:
                    win[p_in, mchunk * 128 + pp * 64: mchunk * 128 + pp * 64 + 64] \
                        += cin_w[:, 0, di, dj]
    win = win.astype(np.float32)
    win4 = np.zeros((128, 2 * 128), np.float32)
    for mchunk in range(8):
        r, g = mchunk // 4, mchunk % 4
        win4[32 * g:32 * g + 16, 128 * r:128 * (r + 1)] = \
            win[:, mchunk * 128:(mchunk + 1) * 128]
    bcin = np.concatenate([cin_b, cin_b]).reshape(128, 1).astype(np.float32)

    E1 = _conv_feature_matrix(c1_w)
    E2 = _conv_feature_matrix(c2_w)
    e1p = _pack_pairs(E1, SC)                                    # (128, 20*256)
    e2p = _pack_pairs(E2, SC)

    # cout in fp16 (fp8 here fails the error budget: the output sum has
    # heavy cancellation, so hh2 quantization noise does not average out)
    Eo = _conv_feature_matrix(cout_w)                            # (1024, 16)
    eout_cat = np.concatenate(
        [Eo[kb * 128:(kb + 1) * 128, :] for kb in range(8)], axis=1
    ).astype(np.float32)                                         # (128, 128)
    bcout = np.repeat(cout_b, 16).reshape(16, 1).astype(np.float32)

    weights = {
        "w1p": W1p.astype(NP16), "b1v": b1.reshape(128, 1),
        "w2": W2.astype(NP16), "b2v": b2p.reshape(128, 1),
        "wg": wg_cat.astype(NP16), "gvec": gvec,
        "win4": win4.astype(NP16), "bcin": bcin,
        "eout": eout_cat.astype(NP16), "bcout": bcout,
        "e1p": e1p.astype(NP8), "e2p": e2p.astype(NP8),
    }
    return x16, z16.astype(NP16), weights


def _make_in_maps(inputs):
    x16, z16, weights = _prepare(inputs)
    in_maps = []
    for k in range(NCORES):
        sl = slice(k * BL, (k + 1) * BL)
        m = {"x16": np.ascontiguousarray(x16[:, sl]),
             "z16": np.ascontiguousarray(z16[:, sl])}
        m.update(weights)
        in_maps.append(m)
    return in_maps


_CACHE = {}

WSPECS = [
    # emission order = DMA order: front weights first, big conv weights last
    ("w1p", [32, 128], F16), ("b1v", [128, 1], F32),
    ("win4", [128, 2 * 128], F16), ("bcin", [128, 1], F32),
    ("w2", [128, 128], F16), ("b2v", [128, 1], F32),
    ("wg", [128, 512], F16), ("gvec", [128, 4], F32),
    ("eout", [128, 128], F16), ("bcout", [16, 1], F32),
    ("e1p", [128, NPAIR * 256], FP8), ("e2p", [128, NPAIR * 256], FP8),
]


def _build_nc(reps=1):
    """Build the single-core Bass program (shared SPMD across the 8 cores)."""
    nc = bacc.Bacc("TRN2", target_bir_lowering=False, debug=False)

    x16_d = nc.dram_tensor("x16", [32, BL], F16, kind="ExternalInput")
    z16_d = nc.dram_tensor("z16", [16, BL], F16, kind="ExternalInput")
    out_d = nc.dram_tensor("out", [16, BL], F32, kind="ExternalOutput")

    wd = {}
    for name, shape, dt in WSPECS:
        wd[name] = nc.dram_tensor(name, shape, dt, kind="ExternalInput")

    with tile.TileContext(nc) as tc:
        with tc.tile_pool(name="wpool", bufs=1) as wp, \
             tc.tile_pool(name="act", bufs=3) as ap, \
             tc.tile_pool(name="ps", bufs=2, space="PSUM") as pp:

            ws = {}
            for name, shape, dt in WSPECS:
                tl = wp.tile(list(wd[name].shape), wd[name].dtype,
                             name=f"sb_{name}")
                nc.sync.dma_start(out=tl, in_=wd[name].ap())
                ws[name] = tl

            def _emit_body():
                d1 = OPTS["front_lead"]      # front runs this far ahead
                d2 = d1 + 1
                st, dma = {}, {}
                dma[0] = _emit_dma(nc, ap, x16_d, z16_d, 0)
                for it in range(NIT + d2):
                    if it < NIT:
                        if it + 1 < NIT:
                            dma[it + 1] = _emit_dma(nc, ap, x16_d, z16_d,
                                                    it + 1)
                        st[it] = _emit_front(nc, ap, pp, ws, *dma.pop(it))
                    if it >= d1 and it - d1 < NIT:
                        s = st[it - d1]
                        s["hh1"] = _conv_layer(
                            nc, ap, pp, ws["e1p"], s["hh0"], s["G1"], s["BB1"],
                            FP8, "c1")
                    if it >= d2 and it - d2 < NIT:
                        s = st.pop(it - d2)
                        hh2 = _conv_layer(nc, ap, pp, ws["e2p"], s["hh1"],
                                          s["G2"], s["BB2"], F16, "c2")
                        _emit_out(nc, ap, pp, ws, out_d, it - d2, hh2)

            if reps == 1:
                _emit_body()
            else:
                ET = mybir.EngineType
                with tc.For_i(0, reps,
                              hint_engines=(ET.PE, ET.Activation, ET.DVE,
                                            ET.SP, ET.Pool)):
                    _emit_body()

    nc.compile()
    return nc


def _elu_shift(nc, ap, pp, ws, pre, bp, tag):
    """h = elu(pre+b)+1 = max(pre+b,0) + min(exp(pre+b),1).

    a on DVE (PSUM), exp on ACT (PSUM, bias folded), merge on Pool (SBUF)."""
    a = ap.tile([128, NT], F16, name=f"{tag}_a", tag="elutmp",
                bufs=OPTS["mlp_bufs"])
    nc.vector.tensor_scalar(a, pre, ws[bp], 0.0, AluOpType.add, AluOpType.max)
    e = ap.tile([128, NT], F16, name=f"{tag}_e", tag="elutmp",
                bufs=OPTS["mlp_bufs"])
    nc.scalar.activation(e, pre, AFT.Exp, bias=ws[bp])
    ec = ap.tile([128, NT], F16, name=f"{tag}_ec", tag="elutmp",
                 bufs=OPTS["mlp_bufs"])
    nc.vector.tensor_scalar(ec, e, 1.0, None, AluOpType.min)
    h = ap.tile([128, NT], F16, name=f"{tag}_h", tag="elutmp",
                bufs=OPTS["mlp_bufs"])
    nc.gpsimd.tensor_tensor(h, ec, a, AluOpType.add)
    return h


def _emit_dma(nc, ap, x16_d, z16_d, it):
    """Prefetch tile `it`'s inputs one pipeline step ahead."""
    sl = slice(it * NT, (it + 1) * NT)
    x_t = ap.tile([32, NT], F16, name="x_t")
    nc.sync.dma_start(out=x_t, in_=x16_d.ap()[:, sl])
    # z replicated at partition rows 0/32/64/96 for row-tiled cin matmuls
    z_t = ap.tile([128, NT], F16, name="z_t")
    for g in range(4):
        nc.sync.dma_start(out=z_t[32 * g:32 * g + 16, :],
                          in_=z16_d.ap()[:, sl])
    return x_t, z_t


def _emit_front(nc, ap, pp, ws, x_t, z_t):
    # ---- MLP layer 1; the cin matmuls fill the PE gap while the ELU
    # chain (DVE/ACT/DVE) produces h1.
    pre1 = pp.tile([128, NT], F32, name="pre1", tag="mg", bufs=OPTS["mg_bufs"])
    nc.tensor.matmul(pre1, ws["w1p"], x_t, start=True, stop=True)
    h1 = _elu_shift(nc, ap, pp, ws, pre1, "b1v", "h1")

    # ---- cin: 8 row-tiled K=16 matmuls (4 concurrent row groups x 2
    # rounds) into 4 chunk-pair psums + tanh into fp8 oct
    hh0 = ap.tile([128, 8 * NT], FP8, name="hh0", tag="hh0",
                  bufs=OPTS["hh_bufs"])
    for r in range(2):
        cps = [pp.tile([128, 2 * NT], F32, name=f"cin_ps{r}{i}", tag="cps",
                       bufs=OPTS["cps_bufs"]) for i in range(2)]
        for g in range(4):
            m = 4 * r + g
            nc.tensor.matmul(
                cps[g // 2][:, (m % 2) * NT:(m % 2 + 1) * NT],
                ws["win4"][32 * g:32 * g + 16, 128 * r:128 * (r + 1)],
                z_t[32 * g:32 * g + 16, :], start=True, stop=True,
                tile_position=(32 * g, 0))
        for i in range(2):
            mp = 2 * r + i
            nc.scalar.activation(hh0[:, mp * 2 * NT:(mp + 1) * 2 * NT],
                                 cps[i], AFT.Tanh, bias=ws["bcin"])

    pre2 = pp.tile([128, NT], F32, name="pre2", tag="mg", bufs=OPTS["mg_bufs"])
    nc.tensor.matmul(pre2, ws["w2"], h1, start=True, stop=True)
    xi = _elu_shift(nc, ap, pp, ws, pre2, "b2v", "h2")

    # ---- gating vectors: matmul into PSUM, evacuate with bias folded.
    # G1/BB1 on DVE, G2/BB2 on ACT (engine balance); G in f32 (exact /SC
    # fold), BB in fp16 so the modulation add keeps the DVE 2x mode.
    gt = []
    for c in range(4):
        gps = pp.tile([128, NT], F32, name=f"gps{c}", tag="mg",
                      bufs=OPTS["mg_bufs"])
        nc.tensor.matmul(gps, ws["wg"][:, c * 128:(c + 1) * 128], xi,
                         start=True, stop=True)
        gsb = ap.tile([128, NT], F32 if c < 2 else F16, name=f"g{c}",
                      tag="gatesG" if c < 2 else "gatesB",
                      bufs=OPTS["gate_bufs"])
        nc.scalar.activation(gsb, gps, AFT.Identity,
                             bias=ws["gvec"][:, c:c + 1])
        gt.append(gsb)
    G1, G2, BB1, BB2 = gt

    return {"hh0": hh0, "G1": G1, "G2": G2, "BB1": BB1, "BB2": BB2}


def _conv_layer(nc, ap, pp, ecat, hin, G, BB, outdt, name):
    """tanh(G * (block matmul of hin) + BB) -> (128, 8*NT) oct, fp8."""
    t1 = ap.tile([128, 8 * NT], F16, name=f"{name}_t1", tag=f"{name}_t1",
                 bufs=OPTS["t_bufs"])
    hout = ap.tile([128, 8 * NT], outdt, name=f"{name}_h", tag=f"{name}_h",
                   bufs=OPTS["hh_bufs"] if name == "c1" else OPTS["t_bufs"])
    Gb = G.unsqueeze(1).broadcast_to((128, 2, NT))
    BBb = BB.unsqueeze(1).broadcast_to((128, 4, NT))
    for mp in range(4):
        cps = pp.tile([128, 2 * NT], F32, name=f"{name}_ps{mp}", tag="cps",
                      bufs=OPTS["cps_bufs"])
        for hf in range(2):
            m = 2 * mp + hf
            prs = PAIRS_OF[m]
            for i, kb1 in enumerate(prs):
                lhsT = ecat[:, PIDX[(m, kb1)] * 256:
                            (PIDX[(m, kb1)] + 1) * 256].rearrange(
                                "p (two f) -> p two f", two=2)
                rhs = hin[:, kb1 * NT:(kb1 + 2) * NT].rearrange(
                    "p (two f) -> p two f", two=2)
                nc.tensor.matmul(cps[:, hf * NT:(hf + 1) * NT],
                                 lhsT, rhs, start=(i == 0),
                                 stop=(i == len(prs) - 1),
                                 perf_mode=PM.DoubleRow)
        # t1 = G * C  (G carries its bias from materialization)
        nc.vector.tensor_tensor(t1[:, mp * 2 * NT:(mp + 1) * 2 * NT],
                                cps, Gb, AluOpType.mult)
    # t1 += BB in place; all-fp16 TT runs in the DVE 2x mode.  The Pool
    # engine is ~4x slower, so it only gets the first quarter (whose conv
    # results land earliest); the tanh halves follow their adds so the
    # next layer can start on half 0 while half 1 is still being added.
    BBq = BB.unsqueeze(1).broadcast_to((128, 2, NT))
    nc.gpsimd.tensor_tensor(t1[:, 0:2 * NT], t1[:, 0:2 * NT], BBq,
                            AluOpType.add)
    nc.vector.tensor_tensor(t1[:, 2 * NT:4 * NT], t1[:, 2 * NT:4 * NT], BBq,
                            AluOpType.add)
    nc.scalar.activation(hout[:, 0:4 * NT], t1[:, 0:4 * NT], AFT.Tanh)
    nc.vector.tensor_tensor(t1[:, 4 * NT:8 * NT], t1[:, 4 * NT:8 * NT], BBb,
                            AluOpType.add)
    nc.scalar.activation(hout[:, 4 * NT:8 * NT], t1[:, 4 * NT:8 * NT],
                         AFT.Tanh)
    return hout


def _emit_out(nc, ap, pp, ws, out_d, it, hh2):
    sl = slice(it * NT, (it + 1) * NT)
    opst = pp.tile([128, NT], F32, name="out_ps", tag="mg",
                   bufs=OPTS["mg_bufs"])
    ops = opst[0:16, :]
    for kb in range(8):
        nc.tensor.matmul(ops, ws["eout"][:, kb * 16:(kb + 1) * 16],
                         hh2[:, kb * NT:(kb + 1) * NT],
                         start=(kb == 0), stop=(kb == 7))
    osb = ap.tile([16, NT], F32, name="osb")
    # cout bias folds into the evacuation (no bias matmul)
    nc.vector.tensor_scalar(osb, ops, ws["bcout"], None, AluOpType.add)
    nc.sync.dma_start(out=out_d.ap()[:, sl], in_=osb)


def kernel(**inputs):
    in_maps = _make_in_maps(inputs)

    if "nc" not in _CACHE:
        _CACHE["nc"] = _build_nc()
    nc = _CACHE["nc"]

    res = run_bass_kernel_spmd(nc, in_maps, core_ids=list(range(NCORES)),
                               trace=bool(_CACHE.get("trace", False)))
    _CACHE["last_results"] = res

    out = np.empty((B, 1, 4, 4), np.float32)
    for k in range(NCORES):
        out[k * BL:(k + 1) * BL] = res.results[k]["out"].T.reshape(BL, 1, 4, 4)
    return out
